# revision 27
# baseline (speedup 1.0000x reference)
"""Trainium2 Bass kernel for nn_BaseNeuralNetworkPotential (pairlist build).

Reference computation (see problem): for the static upper-triangular pair list
(i, j) over N=6000 atoms, return
    pair_indices [2, P] int32   (static triu indices -- input independent)
    d_ij  [P, 1] f32            (pair distance, zeroed where invalid)
    r_ij  [P, 3] f32            (positions[j] - positions[i], zeroed where invalid)
    valid [P]    bool           (same-molecule AND d <= 0.5)

Key structure: atomic_subsystem_indices is sorted, so the valid pairs of row i
form a contiguous *prefix* of the row's output slice: j in [i+1, mol_end).
Molecules have ~60 atoms, so of the 18M pairs only ~180k can be valid, all
within a narrow band of width W (= max molecule size, rounded up) next to the
diagonal.  The kernel therefore:
  1. zero-fills the dense outputs with large (multi-MB) DMAs at full HBM
     bandwidth  (this is the memory-roofline part: ~306MB of output),
  2. computes the W-wide diagonal band (r/d/valid, masked) on-chip and writes
     it as compact [rows, W] tensors,
and the host scatters the compact band over the zero background while
unsharding (pure O(N*W) indexing; all per-pair math happens on device).

Sharding: 16 row-chunks of 375 rows; core c takes chunks (c, 15-c) so each
core owns exactly P/8 = 2,249,625 pairs.  The SPMD program is identical on
every core; per-core behavior comes from host-sliced input tensors.
"""

import numpy as np

import concourse.bacc as bacc
import concourse.bass as bass
import concourse.tile as tile
from concourse import mybir
from concourse.bass_utils import run_bass_kernel_spmd

# ---------------------------------------------------------------- constants
N = 6000          # atoms
NCORES = 8
NCHUNK = 16       # row chunks
CH = N // NCHUNK  # 375 rows per chunk
CHP = 384         # chunk rows padded to 3 tiles of 128
ROWS = 2 * CHP    # partition-rows processed per core (6 tiles of 128)
TILES = ROWS // 128

_L = np.arange(N - 1, -1, -1, dtype=np.int64)          # L[i] = N-1-i
_S = np.zeros(N + 1, dtype=np.int64)
_S[1:] = np.cumsum(_L)                                  # S[i] = first pair of row i
P = int(_S[N])                                          # 17,997,000
PC = P // NCORES                                        # pairs per core
assert P % NCORES == 0
for _c in range(NCORES):
    _pa = int(_S[(_c + 1) * CH] - _S[_c * CH])
    _pb = int(_S[(16 - _c) * CH] - _S[(15 - _c) * CH])
    assert _pa + _pb == PC

_F32 = mybir.dt.float32
_U8 = mybir.dt.uint8

_NC_CACHE: dict[int, "bass.Bass"] = {}
_PAIR_IDX_CACHE: list[np.ndarray] = []


def _pair_indices() -> np.ndarray:
    """Static [2, P] int32 triu indices (input independent)."""
    if not _PAIR_IDX_CACHE:
        counts = _L
        i_idx = np.repeat(np.arange(N, dtype=np.int32), counts)
        j_idx = (
            np.arange(P, dtype=np.int64) - np.repeat(_S[:-1], counts)
        ).astype(np.int32) + i_idx + 1
        _PAIR_IDX_CACHE.append(np.stack([i_idx, j_idx]).astype(np.int32))
    return _PAIR_IDX_CACHE[0]


# ---------------------------------------------------------------- device IR
def _emit_zero_fill(nc, dram_flat, total, zap, zf, engines):
    """Fill dram_flat[0:total] with zeros from the [128, zf] SBUF zero tile.

    Plain [128, k] source patterns only -- step-0 "repeat" dims are lowered
    incorrectly by the hardware DGE (reads stray SBUF), so the max transfer
    per DMA is the full zero tile.  `engines` is cycled per DMA to spread
    issues over both HWDGE rings.
    """
    pstep = zap.ap[0]          # [step, 128] partition dim of the SBUF tile
    assert total >= 128
    per128 = total // 128      # columns when spread over 128 partitions
    pos = 0
    i = 0
    while pos < per128:
        k = min(zf, per128 - pos)
        src = bass.AP(tensor=zap.tensor, offset=zap.offset,
                      ap=[pstep, [1, k]])
        engines[i % len(engines)].dma_start(
            out=dram_flat[pos * 128:(pos + k) * 128], in_=src)
        i += 1
        pos += k
    if total % 128:
        # Cover the sub-128 tail with one more [128, 1] transfer overlapping
        # the already-zeroed region (identical bytes, so overlap is benign).
        # Avoids 1-partition APs, which HW DGE lowers differently than sim.
        src = bass.AP(tensor=zap.tensor, offset=zap.offset,
                      ap=[pstep, [1, 1]])
        engines[i % len(engines)].dma_start(
            out=dram_flat[total - 128:total], in_=src)


def build_nc(w: int) -> "bass.Bass":
    """Build the (core-uniform) SPMD program for band width w."""
    # Bacc (not raw Bass): its compile() legalizes multi-wait instructions
    # (TRN2 allows one semaphore wait per instruction).
    nc = bacc.Bacc(enable_partition_id=False)
    al = mybir.AluOpType

    # [x, y, z, mol_id] per atom.  One sliding-window DMA per tile gives each
    # partition-row its own atom (slot 0) plus its w-atom j-window (slots
    # 1..w).  A single input DMA per tile keeps every compute instruction at
    # <=1 semaphore wait (the big-operand ISA encodings have only one slot).
    win4 = nc.dram_tensor("win4", [2, CHP + w, 4], _F32, kind="ExternalInput")

    out_d = nc.dram_tensor("out_d", [PC], _F32, kind="ExternalOutput")
    out_r = nc.dram_tensor("out_r", [3 * PC], _F32, kind="ExternalOutput")
    out_v = nc.dram_tensor("out_v", [PC], _U8, kind="ExternalOutput")
    # [128, TILES, w]: SBUF accumulator order (partition-major), so the
    # final single DMA per array is a straight contiguous copy; the host
    # transposes back to row order.
    win_d = nc.dram_tensor("win_d", [128, TILES, w], _F32, kind="ExternalOutput")
    win_r = nc.dram_tensor("win_r", [128, TILES, 3 * w], _F32, kind="ExternalOutput")
    win_v = nc.dram_tensor("win_v", [128, TILES, w], _U8, kind="ExternalOutput")

    zf = 8192
    with tile.TileContext(nc) as tc:
        with (
            tc.tile_pool(name="zeros", bufs=1) as zeros_pool,
            tc.tile_pool(name="work", bufs=TILES + 1) as work,
            tc.tile_pool(name="accum", bufs=1) as accum,
        ):
            # two zero tiles: the small one is memset first so the first
            # zero chunks can launch ~5us earlier than one big memset allows.
            zs = 2048
            zbuf_s = zeros_pool.tile([128, zs], _F32)
            nc.vector.memset(zbuf_s[:], 0.0)
            zbuf = zeros_pool.tile([128, zf], _F32)
            nc.vector.memset(zbuf[:], 0.0)
            zap_s = zbuf_s[:]
            zap = zbuf[:]
            # Bulk zeros on the HWDGE/SP ring; the window I/O below runs on
            # SWDGE so its DMA-lane waits never reference these multi-MB
            # chunks (which would FIFO-stall the issuing sequencer).
            rings = [nc.sync]
            head = 4 * 128 * zs           # first 4MB from the small tile
            _emit_zero_fill(nc, out_d[:head], head, zap_s, zs, rings)
            _emit_zero_fill(nc, out_d[head:], PC - head, zap, zf, rings)
            _emit_zero_fill(nc, out_r[:], 3 * PC, zap, zf, rings)
            zap8 = zap.bitcast(_U8)
            _emit_zero_fill(nc, out_v[:], PC, zap8, 4 * zf, rings)

            wd_all = accum.tile([128, TILES, w], _F32)
            wr_all = accum.tile([128, TILES, 3 * w], _F32)
            wv_all = accum.tile([128, TILES, w], _U8)

            w4_ap = win4[:]
            for t in range(TILES):
                chunk, lr = t // 3, (t % 3) * 128
                a, b = t * 128, (t + 1) * 128

                cw = work.tile([128, 4 * (w + 1)], _F32)
                src = bass.AP(tensor=w4_ap.tensor,
                              offset=(chunk * (CHP + w) + lr) * 4,
                              ap=[[4, 128], [1, 4 * (w + 1)]])
                nc.sync.dma_start(out=cw[:], in_=src)
                cw4 = cw[:].rearrange("p (a c) -> p a c", c=4)
                own = cw4[:, 0, :]              # [128, 4] row atom
                wn = cw4[:, 1:w + 1, :]         # [128, w, 4] j-window

                negown = work.tile([128, 4], _F32)
                nc.vector.tensor_scalar_mul(negown[:], own, -1.0)

                r_il = work.tile([128, 3 * w], _F32)
                r3 = r_il[:].rearrange("p (w c) -> p w c", c=3)

                for c_ in range(3):
                    # r_c = pos_j_c - pos_i_c
                    nc.vector.tensor_scalar_add(
                        r3[:, :, c_], wn[:, :, c_], negown[:, c_:c_ + 1])
                sdiff = work.tile([128, w], _F32)
                nc.vector.tensor_scalar_add(
                    sdiff[:], wn[:, :, 3], negown[:, 3:4])

                d2 = work.tile([128, w], _F32)
                tmp = work.tile([128, w], _F32)
                nc.vector.tensor_mul(d2[:], r3[:, :, 0], r3[:, :, 0])
                nc.vector.tensor_mul(tmp[:], r3[:, :, 1], r3[:, :, 1])
                nc.vector.tensor_add(d2[:], d2[:], tmp[:])
                nc.vector.tensor_mul(tmp[:], r3[:, :, 2], r3[:, :, 2])
                nc.vector.tensor_add(d2[:], d2[:], tmp[:])

                inc = work.tile([128, w], _F32)
                nc.vector.tensor_scalar(
                    inc[:], d2[:], 0.25, None, al.is_le)
                valid = work.tile([128, w], _F32)
                # valid = (sdiff == 0) * (d2 <= 0.25)
                nc.vector.scalar_tensor_tensor(
                    valid[:], sdiff[:], 0.0, inc[:], al.is_equal, al.mult)

                # mask before the sqrt (sqrt(0) == 0) so this DVE->ACT chain
                # needs only one cross-engine wait.
                d2m = work.tile([128, w], _F32)
                nc.vector.tensor_mul(d2m[:], d2[:], valid[:])
                y0 = work.tile([128, w], _F32)
                nc.scalar.sqrt(y0[:], d2m[:])
                # one Newton step: d = 0.5*(y0 + d2m/y0) -- the ACT sqrt LUT
                # is only ~3e-6 accurate.  Clamp avoids 0/0 on masked lanes.
                yc = work.tile([128, w], _F32)
                nc.vector.tensor_scalar(yc[:], y0[:], 1e-30, None, al.max)
                nc.vector.reciprocal(yc[:], yc[:])
                nc.vector.tensor_mul(yc[:], d2m[:], yc[:])
                nc.vector.tensor_add(yc[:], yc[:], y0[:])
                nc.vector.tensor_scalar_mul(wd_all[:, t, :], yc[:], 0.5)

                rm3 = wr_all[:, t, :].rearrange("p (w c) -> p w c", c=3)
                for c_ in range(3):
                    nc.vector.tensor_mul(rm3[:, :, c_], r3[:, :, c_], valid[:])

                nc.vector.tensor_copy(out=wv_all[:, t, :], in_=valid[:])

            # SWDGE ring: separate DMA-lane pool, so these don't inherit
            # lane-ordering waits behind the multi-MB zero chunks and their
            # packets interleave mid-phase instead of tailing the kernel.
            nc.gpsimd.dma_start(out=win_d[:], in_=wd_all[:])
            nc.gpsimd.dma_start(out=win_r[:], in_=wr_all[:])
            nc.gpsimd.dma_start(out=win_v[:], in_=wv_all[:])
    nc.compile()
    return nc


def get_nc(w: int) -> "bass.Bass":
    if w not in _NC_CACHE:
        _NC_CACHE[w] = build_nc(w)
    return _NC_CACHE[w]


# ---------------------------------------------------------------- host side
def band_width(sub: np.ndarray) -> int:
    """W = max same-molecule run length, rounded up (>=128, mult of 64)."""
    change = np.flatnonzero(np.diff(sub) != 0)
    starts = np.r_[0, change + 1]
    ends = np.r_[change + 1, len(sub)]
    maxrun = int((ends - starts).max())
    return max(128, int(np.ceil((maxrun - 1) / 64)) * 64)


def make_in_maps(pos: np.ndarray, subf: np.ndarray, w: int):
    """Per-core input tensors.  pos [N,3] f32; subf [N] f32 (molecule ids)."""
    in_maps = []
    for c in range(NCORES):
        chunks = (c, 15 - c)
        # [x, y, z, mol_id] per atom; padding past the last atom gets mol
        # id -1, which no real row (mol >= 0) matches.
        win4 = np.zeros((2, CHP + w, 4), np.float32)
        win4[:, :, 3] = -1.0
        for k, ch in enumerate(chunks):
            g0 = ch * CH
            g1 = min(g0 + CHP + w, N)
            win4[k, :g1 - g0, :3] = pos[g0:g1]
            win4[k, :g1 - g0, 3] = subf[g0:g1]
        in_maps.append({"win4": win4})
    return in_maps


def assemble(results, w: int):
    """Gather per-core outputs into the full arrays."""
    d_full = np.empty(P, np.float32)
    r_full = np.empty((P, 3), np.float32)
    v_full = np.empty(P, np.uint8)
    win_d_all = np.empty((N, w), np.float32)
    win_r_all = np.empty((N, 3 * w), np.float32)
    win_v_all = np.empty((N, w), np.uint8)
    for c in range(NCORES):
        res = results[c]
        r_view = res["out_r"].reshape(PC, 3)
        # [128, TILES, w] accumulator order -> [ROWS, w] row order
        wd_rows = res["win_d"].transpose(1, 0, 2).reshape(ROWS, w)
        wr_rows = res["win_r"].transpose(1, 0, 2).reshape(ROWS, 3 * w)
        wv_rows = res["win_v"].transpose(1, 0, 2).reshape(ROWS, w)
        off = 0
        for k, ch in enumerate((c, 15 - c)):
            p0, p1 = int(_S[ch * CH]), int(_S[(ch + 1) * CH])
            n = p1 - p0
            d_full[p0:p1] = res["out_d"][off:off + n]
            r_full[p0:p1] = r_view[off:off + n]
            v_full[p0:p1] = res["out_v"][off:off + n]
            off += n
            g0 = ch * CH
            win_d_all[g0:g0 + CH] = wd_rows[k * CHP:k * CHP + CH]
            win_r_all[g0:g0 + CH] = wr_rows[k * CHP:k * CHP + CH]
            win_v_all[g0:g0 + CH] = wv_rows[k * CHP:k * CHP + CH]

    # scatter the diagonal band over the zero background
    cols = np.arange(w, dtype=np.int64)[None, :]
    idx = _S[:N, None] + cols                       # [N, w] flat pair index
    mask = cols < _L[:, None]                       # clip rows shorter than w
    flat = idx[mask]
    d_full[flat] = win_d_all[mask]
    r_full[flat] = win_r_all.reshape(N, w, 3)[mask]
    v_full[flat] = win_v_all[mask]
    return (
        _pair_indices(),
        d_full.reshape(P, 1),
        r_full,
        v_full.view(np.bool_),
    )


def _host_fallback(pos: np.ndarray, sub: np.ndarray):
    """Correctness fallback for inputs that violate the sortedness the device
    program relies on.  Not expected to run for spec-compliant inputs."""
    pi = _pair_indices()
    i_idx, j_idx = pi[0], pi[1]
    r = pos[j_idx] - pos[i_idx]
    d = np.sqrt(np.sum(r * r, axis=1, keepdims=True, dtype=np.float32))
    valid = (sub[i_idx] == sub[j_idx]) & (d[:, 0] <= 0.5)
    return (pi, np.where(valid[:, None], d, 0.0).astype(np.float32),
            np.where(valid[:, None], r, 0.0).astype(np.float32), valid)


def kernel(positions, atomic_subsystem_indices):
    pos = np.ascontiguousarray(np.asarray(positions, dtype=np.float32))
    sub = np.asarray(atomic_subsystem_indices)
    assert pos.shape == (N, 3) and sub.shape == (N,)
    if np.any(np.diff(sub) < 0):
        return _host_fallback(pos, sub)
    subf = sub.astype(np.float32)

    w = band_width(subf)
    nc = get_nc(w)
    in_maps = make_in_maps(pos, subf, w)
    res = run_bass_kernel_spmd(nc, in_maps, list(range(NCORES)))
    return assemble(res.results, w)


# revision 32
# speedup vs baseline: 1.1329x; 1.1329x over previous
"""Trainium2 Bass kernel for nn_BaseNeuralNetworkPotential (pairlist build).

Reference computation (see problem): for the static upper-triangular pair list
(i, j) over N=6000 atoms, return
    pair_indices [2, P] int32   (static triu indices -- input independent)
    d_ij  [P, 1] f32            (pair distance, zeroed where invalid)
    r_ij  [P, 3] f32            (positions[j] - positions[i], zeroed where invalid)
    valid [P]    bool           (same-molecule AND d <= 0.5)

Key structure: atomic_subsystem_indices is sorted, so the valid pairs of row i
form a contiguous *prefix* of the row's output slice: j in [i+1, mol_end).
Molecules have ~60 atoms, so of the 18M pairs only ~180k can be valid, all
within a narrow band of width W (= max molecule size, rounded up) next to the
diagonal.  The kernel therefore:
  1. zero-fills the dense outputs with large (multi-MB) DMAs at full HBM
     bandwidth  (this is the memory-roofline part: ~306MB of output),
  2. computes the W-wide diagonal band (r/d/valid, masked) on-chip and ships
     it in three compact accumulator tensors,
and the host scatters the compact band over the zero background while
unsharding (pure O(N*W) indexing; all per-pair math happens on device).

Sharding: 16 row-chunks of 375 rows; core c takes chunks (c, 15-c) so each
core owns exactly P/8 = 2,249,625 pairs.  The SPMD program is identical on
every core; per-core behavior comes from host-sliced input tensors.
"""

import numpy as np

import concourse.bacc as bacc
import concourse.bass as bass
import concourse.tile as tile
from concourse import mybir
from concourse.bass_utils import run_bass_kernel_spmd

# ---------------------------------------------------------------- constants
N = 6000          # atoms
NCORES = 8
NCHUNK = 16       # row chunks
CH = N // NCHUNK  # 375 rows per chunk
CHP = 384         # chunk rows padded to 3 tiles of 128
ROWS = 2 * CHP    # partition-rows processed per core (6 tiles of 128)
TILES = ROWS // 128

_L = np.arange(N - 1, -1, -1, dtype=np.int64)          # L[i] = N-1-i
_S = np.zeros(N + 1, dtype=np.int64)
_S[1:] = np.cumsum(_L)                                  # S[i] = first pair of row i
P = int(_S[N])                                          # 17,997,000
PC = P // NCORES                                        # pairs per core
assert P % NCORES == 0
for _c in range(NCORES):
    _pa = int(_S[(_c + 1) * CH] - _S[_c * CH])
    _pb = int(_S[(16 - _c) * CH] - _S[(15 - _c) * CH])
    assert _pa + _pb == PC

_F32 = mybir.dt.float32
_U8 = mybir.dt.uint8

_NC_CACHE: dict = {}
_PAIR_IDX_CACHE: list[np.ndarray] = []


def _pair_indices() -> np.ndarray:
    """Static [2, P] int32 triu indices (input independent)."""
    if not _PAIR_IDX_CACHE:
        counts = _L
        i_idx = np.repeat(np.arange(N, dtype=np.int32), counts)
        j_idx = (
            np.arange(P, dtype=np.int64) - np.repeat(_S[:-1], counts)
        ).astype(np.int32) + i_idx + 1
        _PAIR_IDX_CACHE.append(np.stack([i_idx, j_idx]).astype(np.int32))
    return _PAIR_IDX_CACHE[0]


# ---------------------------------------------------------------- device IR
def _emit_zero_fill(nc, dram_flat, total, zap, zf, engines):
    """Fill dram_flat[0:total] with zeros from the [128, zf] SBUF zero tile.

    Plain [128, k] source patterns only -- step-0 "repeat" dims are lowered
    incorrectly by the hardware DGE (reads stray SBUF), so the max transfer
    per DMA is the full zero tile.  `engines` is cycled per DMA.
    """
    pstep = zap.ap[0]          # [step, 128] partition dim of the SBUF tile
    assert total >= 128
    per128 = total // 128      # columns when spread over 128 partitions
    pos = 0
    i = 0
    while pos < per128:
        k = min(zf, per128 - pos)
        src = bass.AP(tensor=zap.tensor, offset=zap.offset,
                      ap=[pstep, [1, k]])
        engines[i % len(engines)].dma_start(
            out=dram_flat[pos * 128:(pos + k) * 128], in_=src)
        i += 1
        pos += k
    if total % 128:
        # Cover the sub-128 tail with one more [128, 1] transfer overlapping
        # the already-zeroed region (identical bytes, so overlap is benign).
        # Avoids 1-partition APs, which HW DGE lowers differently than sim.
        src = bass.AP(tensor=zap.tensor, offset=zap.offset,
                      ap=[pstep, [1, 1]])
        engines[i % len(engines)].dma_start(
            out=dram_flat[total - 128:total], in_=src)


def build_nc(w: int, win_ring: str = "scalar", zf: int = 8192) -> "bass.Bass":
    """Build the (core-uniform) SPMD program for band width w."""
    # Bacc (not raw Bass): its compile() legalizes multi-wait instructions
    # (TRN2 allows one semaphore wait per instruction).
    nc = bacc.Bacc(enable_partition_id=False)
    al = mybir.AluOpType

    # [x, y, z, mol_id] per atom.  One sliding-window DMA per tile gives each
    # partition-row its own atom (slot 0) plus its w-atom j-window (slots
    # 1..w).  A single input DMA per tile keeps every compute instruction at
    # <=1 semaphore wait (the big-operand ISA encodings have only one slot).
    win4 = nc.dram_tensor("win4", [2, CHP + w, 4], _F32, kind="ExternalInput")

    out_d = nc.dram_tensor("out_d", [PC], _F32, kind="ExternalOutput")
    out_r = nc.dram_tensor("out_r", [3 * PC], _F32, kind="ExternalOutput")
    out_v = nc.dram_tensor("out_v", [PC], _U8, kind="ExternalOutput")
    # [128, TILES, w]: SBUF accumulator order (partition-major), so the
    # final single DMA per array is a straight contiguous copy; the host
    # transposes back to row order.
    win_d = nc.dram_tensor("win_d", [128, TILES, w], _F32, kind="ExternalOutput")
    win_r = nc.dram_tensor("win_r", [128, TILES, 3 * w], _F32, kind="ExternalOutput")
    win_v = nc.dram_tensor("win_v", [128, TILES, w], _U8, kind="ExternalOutput")

    with tile.TileContext(nc) as tc:
        with (
            tc.tile_pool(name="zeros", bufs=1) as zeros_pool,
            # unique buffers per tile when SBUF allows: no write-after-read
            # waits anywhere in the window pipeline
            tc.tile_pool(name="work",
                         bufs=TILES + 1 if w <= 256 else 4) as work,
            tc.tile_pool(name="accum", bufs=1) as accum,
        ):
            # two zero tiles: the small one is memset first so the first
            # zero chunks can launch ~5us earlier than one big memset allows.
            zs = 2048
            zbuf_s = zeros_pool.tile([128, zs], _F32)
            nc.vector.memset(zbuf_s[:], 0.0)
            zbuf = zeros_pool.tile([128, zf], _F32)
            nc.vector.memset(zbuf[:], 0.0)
            zap_s = zbuf_s[:]
            zap = zbuf[:]
            # Bulk zeros go on the HWDGE/SP ring, whose sequencer issues
            # nothing with compute waits, so the chunks stream back to back.
            rings = [nc.sync]
            head = 6 * 128 * zs           # first 6MB from the small tile
            _emit_zero_fill(nc, out_d[:head], head, zap_s, zs, rings)
            _emit_zero_fill(nc, out_d[head:], PC - head, zap, zf, rings)
            _emit_zero_fill(nc, out_r[:], 3 * PC, zap, zf, rings)
            zap8 = zap.bitcast(_U8)
            _emit_zero_fill(nc, out_v[:], PC, zap8, 4 * zf, rings)

            wd_all = accum.tile([128, TILES, w], _F32)
            wr_all = accum.tile([128, TILES, 3 * w], _F32)
            wv_all = accum.tile([128, TILES, w], _U8)

            w4_ap = win4[:]
            for t in range(TILES):
                chunk, lr = t // 3, (t % 3) * 128

                cw = work.tile([128, 4 * (w + 1)], _F32)
                src = bass.AP(tensor=w4_ap.tensor,
                              offset=(chunk * (CHP + w) + lr) * 4,
                              ap=[[4, 128], [1, 4 * (w + 1)]])
                nc.sync.dma_start(out=cw[:], in_=src)
                cw4 = cw[:].rearrange("p (a c) -> p a c", c=4)
                own = cw4[:, 0, :]              # [128, 4] row atom
                wn = cw4[:, 1:w + 1, :]         # [128, w, 4] j-window

                negown = work.tile([128, 4], _F32)
                nc.vector.tensor_scalar_mul(negown[:], own, -1.0)

                r_il = work.tile([128, 3 * w], _F32)
                r3 = r_il[:].rearrange("p (w c) -> p w c", c=3)

                for c_ in range(3):
                    # r_c = pos_j_c - pos_i_c
                    nc.vector.tensor_scalar_add(
                        r3[:, :, c_], wn[:, :, c_], negown[:, c_:c_ + 1])
                sdiff = work.tile([128, w], _F32)
                nc.vector.tensor_scalar_add(
                    sdiff[:], wn[:, :, 3], negown[:, 3:4])

                d2 = work.tile([128, w], _F32)
                tmp = work.tile([128, w], _F32)
                nc.vector.tensor_mul(d2[:], r3[:, :, 0], r3[:, :, 0])
                nc.vector.tensor_mul(tmp[:], r3[:, :, 1], r3[:, :, 1])
                nc.vector.tensor_add(d2[:], d2[:], tmp[:])
                nc.vector.tensor_mul(tmp[:], r3[:, :, 2], r3[:, :, 2])
                nc.vector.tensor_add(d2[:], d2[:], tmp[:])

                inc = work.tile([128, w], _F32)
                # d2 < 0.25+2^-24 is exactly equivalent to the reference's
                # float32 sqrt(d2) <= 0.5 (single crossover: fl(sqrt(
                # 0.25+2^-25)) == 0.5, fl(sqrt(0.25+2^-24)) > 0.5).
                nc.vector.tensor_scalar(
                    inc[:], d2[:], float(np.float32(0.25 + 2.0**-24)),
                    None, al.is_lt)
                valid = work.tile([128, w], _F32)
                # valid = same_molecule * within_cutoff
                nc.vector.scalar_tensor_tensor(
                    valid[:], sdiff[:], 0.0, inc[:], al.is_equal, al.mult)

                # mask before the sqrt (sqrt(0) == 0) so this DVE->ACT chain
                # needs only one cross-engine wait.
                d2m = work.tile([128, w], _F32)
                nc.vector.tensor_mul(d2m[:], d2[:], valid[:])
                y0 = work.tile([128, w], _F32)
                nc.scalar.sqrt(y0[:], d2m[:])
                # one Newton step: d = 0.5*(y0 + d2m/y0) -- the ACT sqrt LUT
                # is only ~3e-6 accurate.  Clamp avoids 0/0 on masked lanes.
                yc = work.tile([128, w], _F32)
                nc.vector.tensor_scalar(yc[:], y0[:], 1e-30, None, al.max)
                nc.vector.reciprocal(yc[:], yc[:])
                nc.vector.tensor_mul(yc[:], d2m[:], yc[:])
                nc.vector.tensor_add(yc[:], yc[:], y0[:])
                nc.vector.tensor_scalar_mul(wd_all[:, t, :], yc[:], 0.5)

                rm3 = wr_all[:, t, :].rearrange("p (w c) -> p w c", c=3)
                for c_ in range(3):
                    nc.vector.tensor_mul(rm3[:, :, c_], r3[:, :, c_], valid[:])

                nc.vector.tensor_copy(out=wv_all[:, t, :], in_=valid[:])

            # ACT HWDGE ring (a separate queue row from the zero chunks on
            # the SP ring): packets round-robin with the zero chunks, so the
            # window data interleaves mid-phase instead of tailing the
            # kernel.  The ACT sequencer's sqrts are done long before these
            # issue, so nothing stalls behind their waits.
            wring = {"scalar": nc.scalar, "sync": nc.sync,
                     "gpsimd": nc.gpsimd}[win_ring]
            wring.dma_start(out=win_d[:], in_=wd_all[:])
            wring.dma_start(out=win_r[:], in_=wr_all[:])
            wring.dma_start(out=win_v[:], in_=wv_all[:])
    nc.compile()
    return nc


def get_nc(w: int, win_ring: str = "scalar", zf: int = 8192) -> "bass.Bass":
    key = (w, win_ring, zf)
    if key not in _NC_CACHE:
        _NC_CACHE[key] = build_nc(w, win_ring, zf)
    return _NC_CACHE[key]


# ---------------------------------------------------------------- host side
def band_width(sub: np.ndarray) -> int:
    """W = max same-molecule run length, rounded up (>=128, mult of 64)."""
    change = np.flatnonzero(np.diff(sub) != 0)
    starts = np.r_[0, change + 1]
    ends = np.r_[change + 1, len(sub)]
    maxrun = int((ends - starts).max())
    return max(128, int(np.ceil((maxrun - 1) / 64)) * 64)


def make_in_maps(pos: np.ndarray, subf: np.ndarray, w: int):
    """Per-core input tensors.  pos [N,3] f32; subf [N] f32 (molecule ids)."""
    in_maps = []
    for c in range(NCORES):
        chunks = (c, 15 - c)
        # [x, y, z, mol_id] per atom; padding past the last atom gets mol
        # id -1, which no real row (mol >= 0) matches.
        win4 = np.zeros((2, CHP + w, 4), np.float32)
        win4[:, :, 3] = -1.0
        for k, ch in enumerate(chunks):
            g0 = ch * CH
            g1 = min(g0 + CHP + w, N)
            win4[k, :g1 - g0, :3] = pos[g0:g1]
            win4[k, :g1 - g0, 3] = subf[g0:g1]
        in_maps.append({"win4": win4})
    return in_maps


def assemble(results, w: int):
    """Gather per-core outputs into the full arrays."""
    d_full = np.empty(P, np.float32)
    r_full = np.empty((P, 3), np.float32)
    v_full = np.empty(P, np.uint8)
    win_d_all = np.empty((N, w), np.float32)
    win_r_all = np.empty((N, 3 * w), np.float32)
    win_v_all = np.empty((N, w), np.uint8)
    for c in range(NCORES):
        res = results[c]
        r_view = res["out_r"].reshape(PC, 3)
        # [128, TILES, w] accumulator order -> [ROWS, w] row order
        wd_rows = res["win_d"].transpose(1, 0, 2).reshape(ROWS, w)
        wr_rows = res["win_r"].transpose(1, 0, 2).reshape(ROWS, 3 * w)
        wv_rows = res["win_v"].transpose(1, 0, 2).reshape(ROWS, w)
        off = 0
        for k, ch in enumerate((c, 15 - c)):
            p0, p1 = int(_S[ch * CH]), int(_S[(ch + 1) * CH])
            n = p1 - p0
            d_full[p0:p1] = res["out_d"][off:off + n]
            r_full[p0:p1] = r_view[off:off + n]
            v_full[p0:p1] = res["out_v"][off:off + n]
            off += n
            g0 = ch * CH
            win_d_all[g0:g0 + CH] = wd_rows[k * CHP:k * CHP + CH]
            win_r_all[g0:g0 + CH] = wr_rows[k * CHP:k * CHP + CH]
            win_v_all[g0:g0 + CH] = wv_rows[k * CHP:k * CHP + CH]

    # scatter the diagonal band over the zero background
    cols = np.arange(w, dtype=np.int64)[None, :]
    idx = _S[:N, None] + cols                       # [N, w] flat pair index
    mask = cols < _L[:, None]                       # clip rows shorter than w
    flat = idx[mask]
    d_full[flat] = win_d_all[mask]
    r_full[flat] = win_r_all.reshape(N, w, 3)[mask]
    v_full[flat] = win_v_all[mask]
    return (
        _pair_indices(),
        d_full.reshape(P, 1),
        r_full,
        v_full.view(np.bool_),
    )


def _host_fallback(pos: np.ndarray, sub: np.ndarray):
    """Correctness fallback for inputs that violate the sortedness the device
    program relies on.  Not expected to run for spec-compliant inputs."""
    pi = _pair_indices()
    i_idx, j_idx = pi[0], pi[1]
    r = pos[j_idx] - pos[i_idx]
    d = np.sqrt(np.sum(r * r, axis=1, keepdims=True, dtype=np.float32))
    valid = (sub[i_idx] == sub[j_idx]) & (d[:, 0] <= 0.5)
    return (pi, np.where(valid[:, None], d, 0.0).astype(np.float32),
            np.where(valid[:, None], r, 0.0).astype(np.float32), valid)


def kernel(positions, atomic_subsystem_indices):
    pos = np.ascontiguousarray(np.asarray(positions, dtype=np.float32))
    sub = np.asarray(atomic_subsystem_indices)
    assert pos.shape == (N, 3) and sub.shape == (N,)
    if np.any(np.diff(sub) < 0):
        return _host_fallback(pos, sub)
    subf = sub.astype(np.float32)

    w = band_width(subf)
    nc = get_nc(w)
    in_maps = make_in_maps(pos, subf, w)
    res = run_bass_kernel_spmd(nc, in_maps, list(range(NCORES)))
    return assemble(res.results, w)



# revision 34
# speedup vs baseline: 1.1682x; 1.0312x over previous
"""Trainium2 Bass kernel for nn_BaseNeuralNetworkPotential (pairlist build).

Reference computation (see problem): for the static upper-triangular pair list
(i, j) over N=6000 atoms, return
    pair_indices [2, P] int32   (static triu indices -- input independent)
    d_ij  [P, 1] f32            (pair distance, zeroed where invalid)
    r_ij  [P, 3] f32            (positions[j] - positions[i], zeroed where invalid)
    valid [P]    bool           (same-molecule AND d <= 0.5)

Key structure: atomic_subsystem_indices is sorted, so the valid pairs of row i
form a contiguous *prefix* of the row's output slice: j in [i+1, mol_end).
Molecules have ~60 atoms, so of the 18M pairs only ~180k can be valid, all
within a narrow band of width W (= max molecule size, rounded up) next to the
diagonal.  The kernel therefore:
  1. zero-fills the dense outputs with large (multi-MB) DMAs at full HBM
     bandwidth  (this is the memory-roofline part: ~306MB of output),
  2. computes the W-wide diagonal band (r/d/valid, masked) on-chip and ships
     it in three compact accumulator tensors,
and the host scatters the compact band over the zero background while
unsharding (pure O(N*W) indexing; all per-pair math happens on device).

Sharding: 16 row-chunks of 375 rows; core c takes chunks (c, 15-c) so each
core owns exactly P/8 = 2,249,625 pairs.  The SPMD program is identical on
every core; per-core behavior comes from host-sliced input tensors.
"""

import numpy as np

import concourse.bacc as bacc
import concourse.bass as bass
import concourse.tile as tile
from concourse import mybir
from concourse.bass_utils import run_bass_kernel_spmd

# ---------------------------------------------------------------- constants
N = 6000          # atoms
NCORES = 8
NCHUNK = 16       # row chunks
CH = N // NCHUNK  # 375 rows per chunk
CHP = 384         # chunk rows padded to 3 tiles of 128
ROWS = 2 * CHP    # partition-rows processed per core (6 tiles of 128)
TILES = ROWS // 128

_L = np.arange(N - 1, -1, -1, dtype=np.int64)          # L[i] = N-1-i
_S = np.zeros(N + 1, dtype=np.int64)
_S[1:] = np.cumsum(_L)                                  # S[i] = first pair of row i
P = int(_S[N])                                          # 17,997,000
PC = P // NCORES                                        # pairs per core
assert P % NCORES == 0
for _c in range(NCORES):
    _pa = int(_S[(_c + 1) * CH] - _S[_c * CH])
    _pb = int(_S[(16 - _c) * CH] - _S[(15 - _c) * CH])
    assert _pa + _pb == PC

_F32 = mybir.dt.float32
_U8 = mybir.dt.uint8

_NC_CACHE: dict = {}
_PAIR_IDX_CACHE: list[np.ndarray] = []


def _pair_indices() -> np.ndarray:
    """Static [2, P] int32 triu indices (input independent)."""
    if not _PAIR_IDX_CACHE:
        counts = _L
        i_idx = np.repeat(np.arange(N, dtype=np.int32), counts)
        j_idx = (
            np.arange(P, dtype=np.int64) - np.repeat(_S[:-1], counts)
        ).astype(np.int32) + i_idx + 1
        _PAIR_IDX_CACHE.append(np.stack([i_idx, j_idx]).astype(np.int32))
    return _PAIR_IDX_CACHE[0]


# ---------------------------------------------------------------- device IR
def _emit_zero_fill(nc, dram_flat, total, zap, zf, engines):
    """Fill dram_flat[0:total] with zeros from the [128, zf] SBUF zero tile.

    Plain [128, k] source patterns only -- step-0 "repeat" dims are lowered
    incorrectly by the hardware DGE (reads stray SBUF), so the max transfer
    per DMA is the full zero tile.  `engines` is cycled per DMA.
    """
    pstep = zap.ap[0]          # [step, 128] partition dim of the SBUF tile
    assert total >= 128
    per128 = total // 128      # columns when spread over 128 partitions
    pos = 0
    i = 0
    while pos < per128:
        k = min(zf, per128 - pos)
        src = bass.AP(tensor=zap.tensor, offset=zap.offset,
                      ap=[pstep, [1, k]])
        engines[i % len(engines)].dma_start(
            out=dram_flat[pos * 128:(pos + k) * 128], in_=src)
        i += 1
        pos += k
    if total % 128:
        # Cover the sub-128 tail with one more [128, 1] transfer overlapping
        # the already-zeroed region (identical bytes, so overlap is benign).
        # Avoids 1-partition APs, which HW DGE lowers differently than sim.
        src = bass.AP(tensor=zap.tensor, offset=zap.offset,
                      ap=[pstep, [1, 1]])
        engines[i % len(engines)].dma_start(
            out=dram_flat[total - 128:total], in_=src)


def build_nc(w: int, win_ring: str = "scalar", zf: int = 8192) -> "bass.Bass":
    """Build the (core-uniform) SPMD program for band width w."""
    # Bacc (not raw Bass): its compile() legalizes multi-wait instructions
    # (TRN2 allows one semaphore wait per instruction).
    nc = bacc.Bacc(enable_partition_id=False)
    al = mybir.AluOpType

    # [x, y, z, mol_id] per atom.  One sliding-window DMA per tile gives each
    # partition-row its own atom (slot 0) plus its w-atom j-window (slots
    # 1..w).  A single input DMA per tile keeps every compute instruction at
    # <=1 semaphore wait (the big-operand ISA encodings have only one slot).
    win4 = nc.dram_tensor("win4", [2, CHP + w, 4], _F32, kind="ExternalInput")

    out_d = nc.dram_tensor("out_d", [PC], _F32, kind="ExternalOutput")
    out_r = nc.dram_tensor("out_r", [3 * PC], _F32, kind="ExternalOutput")
    out_v = nc.dram_tensor("out_v", [PC], _U8, kind="ExternalOutput")
    # [128, TILES, w]: SBUF accumulator order (partition-major), so the
    # final single DMA per array is a straight contiguous copy; the host
    # transposes back to row order.
    win_d = nc.dram_tensor("win_d", [128, TILES, w], _F32, kind="ExternalOutput")
    win_r = nc.dram_tensor("win_r", [128, TILES, 3 * w], _F32, kind="ExternalOutput")
    win_v = nc.dram_tensor("win_v", [128, TILES, w], _U8, kind="ExternalOutput")

    with tile.TileContext(nc) as tc:
        with (
            tc.tile_pool(name="zeros", bufs=1) as zeros_pool,
            # unique buffers per tile when SBUF allows: no write-after-read
            # waits anywhere in the window pipeline
            tc.tile_pool(name="work",
                         bufs=TILES + 1 if w <= 256 else 4) as work,
            tc.tile_pool(name="accum", bufs=1) as accum,
        ):
            # two zero tiles: the small one is memset first so the first
            # zero chunks can launch ~5us earlier than one big memset allows.
            zs = 2048
            zbuf_s = zeros_pool.tile([128, zs], _F32)
            nc.vector.memset(zbuf_s[:], 0.0)
            zbuf = zeros_pool.tile([128, zf], _F32)
            nc.vector.memset(zbuf[:], 0.0)
            zap_s = zbuf_s[:]
            zap = zbuf[:]
            # Bulk zeros go on the HWDGE/SP ring, whose sequencer issues
            # nothing with compute waits, so the chunks stream back to back.
            rings = [nc.sync]
            head = 6 * 128 * zs           # first 6MB from the small tile
            _emit_zero_fill(nc, out_d[:head], head, zap_s, zs, rings)
            _emit_zero_fill(nc, out_d[head:], PC - head, zap, zf, rings)
            _emit_zero_fill(nc, out_r[:], 3 * PC, zap, zf, rings)
            zap8 = zap.bitcast(_U8)
            _emit_zero_fill(nc, out_v[:], PC, zap8, 4 * zf, rings)

            wd_all = accum.tile([128, TILES, w], _F32)
            wr_all = accum.tile([128, TILES, 3 * w], _F32)
            wv_all = accum.tile([128, TILES, w], _U8)

            w4_ap = win4[:]
            for t in range(TILES):
                chunk, lr = t // 3, (t % 3) * 128

                cw = work.tile([128, 4 * (w + 1)], _F32)
                src = bass.AP(tensor=w4_ap.tensor,
                              offset=(chunk * (CHP + w) + lr) * 4,
                              ap=[[4, 128], [1, 4 * (w + 1)]])
                nc.sync.dma_start(out=cw[:], in_=src)
                cw4 = cw[:].rearrange("p (a c) -> p a c", c=4)
                own = cw4[:, 0, :]              # [128, 4] row atom
                wn = cw4[:, 1:w + 1, :]         # [128, w, 4] j-window

                negown = work.tile([128, 4], _F32)
                nc.vector.tensor_scalar_mul(negown[:], own, -1.0)

                r_il = work.tile([128, 3 * w], _F32)
                r3 = r_il[:].rearrange("p (w c) -> p w c", c=3)

                for c_ in range(3):
                    # r_c = pos_j_c - pos_i_c
                    nc.vector.tensor_scalar_add(
                        r3[:, :, c_], wn[:, :, c_], negown[:, c_:c_ + 1])
                sdiff = work.tile([128, w], _F32)
                nc.vector.tensor_scalar_add(
                    sdiff[:], wn[:, :, 3], negown[:, 3:4])

                d2 = work.tile([128, w], _F32)
                tmp = work.tile([128, w], _F32)
                nc.vector.tensor_mul(d2[:], r3[:, :, 0], r3[:, :, 0])
                nc.vector.tensor_mul(tmp[:], r3[:, :, 1], r3[:, :, 1])
                nc.vector.tensor_add(d2[:], d2[:], tmp[:])
                nc.vector.tensor_mul(tmp[:], r3[:, :, 2], r3[:, :, 2])
                nc.vector.tensor_add(d2[:], d2[:], tmp[:])

                inc = work.tile([128, w], _F32)
                # d2 < 0.25+2^-24 is exactly equivalent to the reference's
                # float32 sqrt(d2) <= 0.5 (single crossover: fl(sqrt(
                # 0.25+2^-25)) == 0.5, fl(sqrt(0.25+2^-24)) > 0.5).
                nc.vector.tensor_scalar(
                    inc[:], d2[:], float(np.float32(0.25 + 2.0**-24)),
                    None, al.is_lt)
                valid = work.tile([128, w], _F32)
                # valid = same_molecule * within_cutoff
                nc.vector.scalar_tensor_tensor(
                    valid[:], sdiff[:], 0.0, inc[:], al.is_equal, al.mult)

                # mask before the sqrt (sqrt(0) == 0) so this DVE->ACT chain
                # needs only one cross-engine wait.
                d2m = work.tile([128, w], _F32)
                nc.vector.tensor_mul(d2m[:], d2[:], valid[:])
                y0 = work.tile([128, w], _F32)
                nc.scalar.sqrt(y0[:], d2m[:])
                # one Newton step: d = 0.5*(y0 + d2m/y0) -- the ACT sqrt LUT
                # is only ~3e-6 accurate.  Clamp avoids 0/0 on masked lanes.
                yc = work.tile([128, w], _F32)
                nc.vector.tensor_scalar(yc[:], y0[:], 1e-30, None, al.max)
                nc.vector.reciprocal(yc[:], yc[:])
                nc.vector.tensor_mul(yc[:], d2m[:], yc[:])
                nc.vector.tensor_add(yc[:], yc[:], y0[:])
                nc.vector.tensor_scalar_mul(wd_all[:, t, :], yc[:], 0.5)

                rm3 = wr_all[:, t, :].rearrange("p (w c) -> p w c", c=3)
                for c_ in range(3):
                    nc.vector.tensor_mul(rm3[:, :, c_], r3[:, :, c_], valid[:])

                nc.vector.tensor_copy(out=wv_all[:, t, :], in_=valid[:])

            # ACT HWDGE ring (a separate queue row from the zero chunks on
            # the SP ring): packets round-robin with the zero chunks, so the
            # window data interleaves mid-phase instead of tailing the
            # kernel.  The ACT sequencer's sqrts are done long before these
            # issue, so nothing stalls behind their waits.
            wring = {"scalar": nc.scalar, "sync": nc.sync,
                     "gpsimd": nc.gpsimd}[win_ring]
            wring.dma_start(out=win_d[:], in_=wd_all[:])
            wring.dma_start(out=win_r[:], in_=wr_all[:])
            wring.dma_start(out=win_v[:], in_=wv_all[:])
    nc.compile()
    return nc


def get_nc(w: int, win_ring: str = "scalar", zf: int = 8192) -> "bass.Bass":
    key = (w, win_ring, zf)
    if key not in _NC_CACHE:
        _NC_CACHE[key] = build_nc(w, win_ring, zf)
    return _NC_CACHE[key]


# ---------------------------------------------------------------- host side
def band_width(sub: np.ndarray) -> int:
    """W = max same-molecule run length, rounded up (>=128, mult of 64)."""
    change = np.flatnonzero(np.diff(sub) != 0)
    starts = np.r_[0, change + 1]
    ends = np.r_[change + 1, len(sub)]
    maxrun = int((ends - starts).max())
    return max(128, int(np.ceil((maxrun - 1) / 64)) * 64)


def make_in_maps(pos: np.ndarray, subf: np.ndarray, w: int):
    """Per-core input tensors.  pos [N,3] f32; subf [N] f32 (molecule ids)."""
    in_maps = []
    for c in range(NCORES):
        chunks = (c, 15 - c)
        # [x, y, z, mol_id] per atom; padding past the last atom gets mol
        # id -1, which no real row (mol >= 0) matches.
        win4 = np.zeros((2, CHP + w, 4), np.float32)
        win4[:, :, 3] = -1.0
        for k, ch in enumerate(chunks):
            g0 = ch * CH
            g1 = min(g0 + CHP + w, N)
            win4[k, :g1 - g0, :3] = pos[g0:g1]
            win4[k, :g1 - g0, 3] = subf[g0:g1]
        in_maps.append({"win4": win4})
    return in_maps


def assemble(results, w: int):
    """Gather per-core outputs into the full arrays."""
    d_full = np.empty(P, np.float32)
    r_full = np.empty((P, 3), np.float32)
    v_full = np.empty(P, np.uint8)
    win_d_all = np.empty((N, w), np.float32)
    win_r_all = np.empty((N, 3 * w), np.float32)
    win_v_all = np.empty((N, w), np.uint8)
    for c in range(NCORES):
        res = results[c]
        r_view = res["out_r"].reshape(PC, 3)
        # [128, TILES, w] accumulator order -> [ROWS, w] row order
        wd_rows = res["win_d"].transpose(1, 0, 2).reshape(ROWS, w)
        wr_rows = res["win_r"].transpose(1, 0, 2).reshape(ROWS, 3 * w)
        wv_rows = res["win_v"].transpose(1, 0, 2).reshape(ROWS, w)
        off = 0
        for k, ch in enumerate((c, 15 - c)):
            p0, p1 = int(_S[ch * CH]), int(_S[(ch + 1) * CH])
            n = p1 - p0
            d_full[p0:p1] = res["out_d"][off:off + n]
            r_full[p0:p1] = r_view[off:off + n]
            v_full[p0:p1] = res["out_v"][off:off + n]
            off += n
            g0 = ch * CH
            win_d_all[g0:g0 + CH] = wd_rows[k * CHP:k * CHP + CH]
            win_r_all[g0:g0 + CH] = wr_rows[k * CHP:k * CHP + CH]
            win_v_all[g0:g0 + CH] = wv_rows[k * CHP:k * CHP + CH]

    # scatter the diagonal band over the zero background
    cols = np.arange(w, dtype=np.int64)[None, :]
    idx = _S[:N, None] + cols                       # [N, w] flat pair index
    mask = cols < _L[:, None]                       # clip rows shorter than w
    flat = idx[mask]
    d_full[flat] = win_d_all[mask]
    r_full[flat] = win_r_all.reshape(N, w, 3)[mask]
    v_full[flat] = win_v_all[mask]
    return (
        _pair_indices(),
        d_full.reshape(P, 1),
        r_full,
        v_full.view(np.bool_),
    )


def _host_fallback(pos: np.ndarray, sub: np.ndarray):
    """Correctness fallback for inputs that violate the sortedness the device
    program relies on.  Not expected to run for spec-compliant inputs."""
    pi = _pair_indices()
    i_idx, j_idx = pi[0], pi[1]
    r = pos[j_idx] - pos[i_idx]
    d = np.sqrt(np.sum(r * r, axis=1, keepdims=True, dtype=np.float32))
    valid = (sub[i_idx] == sub[j_idx]) & (d[:, 0] <= 0.5)
    return (pi, np.where(valid[:, None], d, 0.0).astype(np.float32),
            np.where(valid[:, None], r, 0.0).astype(np.float32), valid)


def kernel(positions, atomic_subsystem_indices):
    pos = np.ascontiguousarray(np.asarray(positions, dtype=np.float32))
    sub = np.asarray(atomic_subsystem_indices)
    assert pos.shape == (N, 3) and sub.shape == (N,)
    if np.any(np.diff(sub) < 0):
        return _host_fallback(pos, sub)
    subf = sub.astype(np.float32)

    w = band_width(subf)
    nc = get_nc(w)
    in_maps = make_in_maps(pos, subf, w)
    res = run_bass_kernel_spmd(nc, in_maps, list(range(NCORES)))
    return assemble(res.results, w)

